# revision 15
# baseline (speedup 1.0000x reference)
"""ContextualAttentionMask Trainium2 kernel.

Math (per batch sample):
  f: [256, 4096] feature map (channels x pixels), m: [4096] mask
  K[j, :]    = f[:, j] + 1e-7          (per-pixel 1x1 kernel)
  rstd[j]    = 1 / ||K[j, :]||_2
  raw[j, n]  = sum_c f[c, j] * f[c, n]          (only interior columns matter:
               the conv padding columns are dead compute - 1x1 kernels, the
               output at pad positions is cropped, softmax is per-column)
  att[j, n]  = softmax_j(rstd[j] * raw[j, n])
  fmap[c, n] = sum_j rstd[j] * m[j] * K[j, c] * att[j, n]
  final      = fmap * (1 - m) + f * m  ;  skip branch if mask nearly all-ones

Device computes (per core, unnormalized; host divides, blends, skip-branch):
  E[j, n] = exp(rstd[j] * raw[j, n] - 12)       (-12 keeps E in fp16 range;
                                                 cancels in the division)
  o[c, n] = sum_j km16[j, c] * E[j, n]     with km16 = fp16(rstd * m * K)
  s[n]    = sum_j E[j, n]

Sharding: 8 cores = 4 samples x 2 column-halves (2048 columns each).
Inputs are host-permuted so each core's own half is always columns 0..2047;
the j (softmax/contraction) order is irrelevant as long as f16/km16/rstd
agree. Tiny per-j scalars (rstd, rstd*m) and fp16 casts are host-side prep;
all heavy compute (2x 2048x4096x256 GEMMs + softmax) runs on device.
"""

import sys
from contextlib import ExitStack

import numpy as np

sys.path.insert(0, "/opt/trn_rl_repo")

from concourse import bacc, mybir, tile  # noqa: E402
from concourse.bass_utils import run_bass_kernel_spmd  # noqa: E402

FP32 = mybir.dt.float32
FP16 = mybir.dt.float16

CH = 256          # channels
J = 4096          # number of per-pixel kernels (= h*w)
NH = 2048         # columns handled per core (half of a sample)
EXP_BIAS = -12.0  # exp(x - 12) keeps values in fp16 range; cancels on host


def build_program(ch=CH, j_total=J, n_half=NH, bufs_sc=4, bufs_out=4,
                  bufs_e=10):
    """Emit the per-core Bass/Tile program (SPMD across 8 cores)."""
    assert ch % 128 == 0 and j_total % 128 == 0
    n_cb = ch // 128          # channel blocks
    n_jb = j_total // 128     # j blocks
    qs = min(512, n_half)     # output column chunk width
    nq = n_half // qs
    assert n_half % qs == 0

    nc = bacc.Bacc("TRN2", target_bir_lowering=False, debug=False, num_devices=8)

    f_d = nc.dram_tensor("f16", [ch, j_total], FP16, kind="ExternalInput").ap()
    km_d = nc.dram_tensor("km16", [j_total, ch], FP16, kind="ExternalInput").ap()
    rstd_d = nc.dram_tensor("rstd", [128, n_jb], FP32, kind="ExternalInput").ap()
    o_d = nc.dram_tensor("o", [ch, n_half], FP32, kind="ExternalOutput").ap()
    s_d = nc.dram_tensor("s", [1, n_half], FP32, kind="ExternalOutput").ap()

    with tile.TileContext(nc) as tc, ExitStack() as ctx:
        const_p = ctx.enter_context(tc.tile_pool(name="const", bufs=1))
        kt_p = ctx.enter_context(tc.tile_pool(name="kt", bufs=n_cb))
        km_p = ctx.enter_context(tc.tile_pool(name="km", bufs=n_jb))
        e_p = ctx.enter_context(tc.tile_pool(name="e", bufs=bufs_e))
        osb_p = ctx.enter_context(tc.tile_pool(name="osb", bufs=3))
        ssb_p = ctx.enter_context(tc.tile_pool(name="ssb", bufs=2))
        ps_sc = ctx.enter_context(
            tc.tile_pool(name="ps_sc", bufs=bufs_sc, space="PSUM"))
        ps_out = ctx.enter_context(
            tc.tile_pool(name="ps_out", bufs=bufs_out, space="PSUM"))

        ones32 = const_p.tile([128, 1], FP32)
        nc.vector.memset(ones32[:], 1.0)
        bias_e = const_p.tile([128, 1], FP32, tag="bias_e")
        nc.vector.memset(bias_e[:], EXP_BIAS)
        rstd = const_p.tile([128, n_jb], FP32, tag="rstd")
        nc.sync.dma_start(out=rstd[:], in_=rstd_d[:, :])

        # fp16 feature map, [c, j] layout; chunked DMA so matmuls start early
        kt = [
            kt_p.tile([128, j_total], FP16, tag="kt", name=f"kt{cb}")
            for cb in range(n_cb)
        ]
        cw = min(1024, j_total)
        for q8 in range(0, j_total, cw):
            for cb in range(n_cb):
                nc.sync.dma_start(
                    out=kt[cb][:, q8:q8 + cw],
                    in_=f_d[cb * 128:(cb + 1) * 128, q8:q8 + cw],
                )

        # mask-and-norm-scaled kernels, [j, c] layout
        km = []
        for jb in range(n_jb):
            t = km_p.tile([128, ch], FP16, tag="km", name=f"km{jb}")
            nc.sync.dma_start(out=t[:], in_=km_d[jb * 128:(jb + 1) * 128, :])
            km.append(t)

        # fused main loop: scores -> exp -> sumexp & Km^T E accumulation.
        # The softmax denominator is folded partition-wise on the (idle) DVE
        # (acc[p, n] = sum_jb E[jb*128+p, n]); one fp32 ones-matmul per chunk
        # does the final 128-way fold, keeping the PE stream count minimal.
        for q in range(nq):
            nsl = slice(q * qs, (q + 1) * qs)
            sum_ps = ps_out.tile([1, qs], FP32, tag="out", name="sum_ps")
            acc = ssb_p.tile([128, qs], FP32, tag="acc", name="acc")
            out_ps = [
                ps_out.tile([128, qs], FP32, tag="out", name=f"out_ps{cb}")
                for cb in range(n_cb)
            ]
            for jb in range(n_jb):
                jsl = slice(jb * 128, (jb + 1) * 128)
                ps = ps_sc.tile([128, qs], FP32, tag="sc", name="ps")
                for cb in range(n_cb):
                    nc.tensor.matmul(
                        ps[:], kt[cb][:, jsl], kt[cb][:, nsl],
                        start=(cb == 0), stop=(cb == n_cb - 1),
                    )
                e = e_p.tile([128, qs], FP16, tag="e", name="e")
                nc.scalar.activation(
                    e[:], ps[:], mybir.ActivationFunctionType.Exp,
                    bias=bias_e[:], scale=rstd[:, jb:jb + 1],
                )
                if jb == 0:
                    nc.vector.tensor_copy(acc[:], e[:])
                else:
                    nc.vector.tensor_add(acc[:], acc[:], e[:])
                for cb in range(n_cb):
                    nc.tensor.matmul(
                        out_ps[cb][:], km[jb][:, cb * 128:(cb + 1) * 128], e[:],
                        start=(jb == 0), stop=(jb == n_jb - 1),
                    )
            nc.tensor.matmul(sum_ps[:], ones32[:], acc[:], start=True, stop=True)
            srow = ssb_p.tile([1, qs], FP32, tag="srow", name="srow")
            nc.vector.tensor_copy(srow[:], sum_ps[:])
            nc.sync.dma_start(out=s_d[0:1, nsl], in_=srow[:])
            for cb in range(n_cb):
                osb = osb_p.tile([128, qs], FP32, tag="osb", name="osb")
                nc.vector.tensor_copy(osb[:], out_ps[cb][:])
                nc.sync.dma_start(out=o_d[cb * 128:(cb + 1) * 128, nsl], in_=osb[:])

    nc.compile()
    return nc


_CACHE = {}


def _get_program():
    if "nc" not in _CACHE:
        _CACHE["nc"] = build_program()
    return _CACHE["nc"]


def make_in_maps(foreground, mask):
    """Per-core host-side input prep (permute so own half is first)."""
    bs, ch, h, w = foreground.shape
    hw = h * w
    half = hw // 2
    f = np.ascontiguousarray(foreground.reshape(bs, ch, hw), dtype=np.float32)
    m = np.ascontiguousarray(mask.reshape(bs, hw), dtype=np.float64)
    in_maps = []
    for b in range(bs):
        k = f[b].astype(np.float64) + 1e-7          # [ch, hw], reference's +1e-7
        rstd = 1.0 / np.sqrt((k * k).sum(axis=0))   # [hw]
        km = (rstd * m[b])[:, None] * k.T           # [hw, ch]
        for hh in range(2):
            if hh == 0:
                perm = np.arange(hw)
            else:
                perm = np.concatenate([np.arange(half, hw), np.arange(half)])
            in_maps.append({
                "f16": np.ascontiguousarray(f[b][:, perm]).astype(np.float16),
                "km16": np.ascontiguousarray(km[perm]).astype(np.float16),
                "rstd": np.ascontiguousarray(
                    rstd[perm].reshape(hw // 128, 128).T
                ).astype(np.float32),
            })
    return in_maps


def kernel(foreground, mask):
    foreground = np.asarray(foreground, dtype=np.float32)
    mask = np.asarray(mask, dtype=np.float32)
    bs, ch, h, w = foreground.shape
    hw = h * w

    nc = _get_program()
    in_maps = make_in_maps(foreground, mask)
    res = run_bass_kernel_spmd(nc, in_maps, list(range(8)))

    fmap = np.empty((bs, ch, h, w), dtype=np.float32)
    rows = h // 2
    for core in range(8):
        b, hh = core // 2, core % 2
        o = res.results[core]["o"]       # [ch, hw/2] unnormalized
        s = res.results[core]["s"]       # [1, hw/2] softmax denominator
        fmap[b, :, hh * rows:(hh + 1) * rows, :] = (o / s).reshape(ch, rows, w)

    mm = mask[:, 0:1]                    # [bs, 1, h, w]
    final = fmap * (1.0 - mm) + foreground * mm
    skip = mask.sum(axis=(1, 2, 3)) > (hw - 10)
    final[skip] = foreground[skip]
    return final.astype(np.float32)


# revision 22
# speedup vs baseline: 624.0781x; 624.0781x over previous
"""ContextualAttentionMask Trainium2 kernel.

Math (per batch sample):
  f: [256, 4096] feature map (channels x pixels), m: [4096] mask
  K[j, :]    = f[:, j] + 1e-7          (per-pixel 1x1 kernel)
  rstd[j]    = 1 / ||K[j, :]||_2
  raw[j, n]  = sum_c f[c, j] * f[c, n]          (only interior columns matter:
               the conv padding columns are dead compute - 1x1 kernels, the
               output at pad positions is cropped, softmax is per-column)
  att[j, n]  = softmax_j(rstd[j] * raw[j, n])
  fmap[c, n] = sum_j rstd[j] * m[j] * K[j, c] * att[j, n]
  final      = fmap * (1 - m) + f * m  ;  skip branch if mask nearly all-ones

Device computes (per core, unnormalized; host divides, blends, skip-branch):
  E[j, n] = exp(rstd[j] * raw[j, n] - 12)       (-12 keeps E in fp16 range;
                                                 cancels in the division)
  o[c, n] = sum_j km16[j, c] * E[j, n]     with km16 = fp16(rstd * m * K)
  s[n]    = sum_j E[j, n]

Sharding: 8 cores = 4 samples x 2 column-halves (2048 columns each).
Inputs are host-permuted so each core's own half is always columns 0..2047;
the j (softmax/contraction) order is irrelevant as long as f16/km16/rstd
agree. Tiny per-j scalars (rstd, rstd*m) and fp16 casts are host-side prep;
all heavy compute (2x 2048x4096x256 GEMMs + softmax) runs on device.
"""

import sys
from contextlib import ExitStack

import numpy as np

sys.path.insert(0, "/opt/trn_rl_repo")

from concourse import bacc, mybir, tile  # noqa: E402
from concourse.bass_utils import run_bass_kernel_spmd  # noqa: E402

FP32 = mybir.dt.float32
FP16 = mybir.dt.float16

CH = 256          # channels
J = 4096          # number of per-pixel kernels (= h*w)
NH = 2048         # columns handled per core (half of a sample)
EXP_BIAS = -12.0  # exp(x - 12) keeps values in fp16 range; cancels on host


def build_program(ch=CH, j_total=J, n_half=NH, bufs_sc=4, bufs_out=4,
                  bufs_e=10, loop_reps=1):
    """Emit the per-core Bass/Tile program (SPMD across 8 cores)."""
    assert ch % 128 == 0 and j_total % 128 == 0
    n_cb = ch // 128          # channel blocks
    n_jb = j_total // 128     # j blocks
    qs = min(512, n_half)     # output column chunk width
    nq = n_half // qs
    assert n_half % qs == 0

    nc = bacc.Bacc("TRN2", target_bir_lowering=False, debug=False, num_devices=8)

    f_d = nc.dram_tensor("f16", [ch, j_total], FP16, kind="ExternalInput").ap()
    km_d = nc.dram_tensor("km16", [j_total, ch], FP16, kind="ExternalInput").ap()
    rstd_d = nc.dram_tensor("rstd", [128, n_jb], FP32, kind="ExternalInput").ap()
    o_d = nc.dram_tensor("o", [ch, n_half], FP32, kind="ExternalOutput").ap()
    s_d = nc.dram_tensor("s", [1, n_half], FP32, kind="ExternalOutput").ap()

    with tile.TileContext(nc) as tc, ExitStack() as ctx:
        const_p = ctx.enter_context(tc.tile_pool(name="const", bufs=1))
        kt_p = ctx.enter_context(tc.tile_pool(name="kt", bufs=n_cb))
        km_p = ctx.enter_context(tc.tile_pool(name="km", bufs=n_jb))
        e_p = ctx.enter_context(tc.tile_pool(name="e", bufs=bufs_e))
        osb_p = ctx.enter_context(tc.tile_pool(name="osb", bufs=3))
        ssb_p = ctx.enter_context(tc.tile_pool(name="ssb", bufs=2))
        ps_sc = ctx.enter_context(
            tc.tile_pool(name="ps_sc", bufs=bufs_sc, space="PSUM"))
        ps_out = ctx.enter_context(
            tc.tile_pool(name="ps_out", bufs=bufs_out, space="PSUM"))

        ones32 = const_p.tile([128, 1], FP32)
        nc.vector.memset(ones32[:], 1.0)
        bias_e = const_p.tile([128, 1], FP32, tag="bias_e")
        nc.vector.memset(bias_e[:], EXP_BIAS)
        rstd = const_p.tile([128, n_jb], FP32, tag="rstd")
        nc.sync.dma_start(out=rstd[:], in_=rstd_d[:, :])

        # fp16 feature map, [c, j] layout; chunked DMA so matmuls start early
        kt = [
            kt_p.tile([128, j_total], FP16, tag="kt", name=f"kt{cb}")
            for cb in range(n_cb)
        ]
        if j_total >= 4096:  # small first chunks so the first matmuls start early
            bounds = [0, 512, 1024, 2048, j_total]
        else:
            bounds = list(range(0, j_total + 1, min(512, j_total)))
        for q8, q9 in zip(bounds[:-1], bounds[1:]):
            for cb in range(n_cb):
                nc.sync.dma_start(
                    out=kt[cb][:, q8:q9],
                    in_=f_d[cb * 128:(cb + 1) * 128, q8:q9],
                )

        # mask-and-norm-scaled kernels, [j, c] layout
        km = []
        for jb in range(n_jb):
            t = km_p.tile([128, ch], FP16, tag="km", name=f"km{jb}")
            nc.sync.dma_start(out=t[:], in_=km_d[jb * 128:(jb + 1) * 128, :])
            km.append(t)

        # fused main loop: scores -> exp -> sumexp & Km^T E accumulation.
        # The softmax denominator is folded partition-wise on the (idle) DVE
        # (acc[p, n] = sum_jb E[jb*128+p, n]); one fp32 ones-matmul per chunk
        # does the final 128-way fold, keeping the PE stream count minimal.
        # loop_reps > 1 repeats the identical work (timing experiments only).
        for q in [qq for _ in range(loop_reps) for qq in range(nq)]:
            nsl = slice(q * qs, (q + 1) * qs)
            sum_ps = ps_out.tile([1, qs], FP32, tag="out", name="sum_ps")
            acc = ssb_p.tile([128, qs], FP32, tag="acc", name="acc")
            out_ps = [
                ps_out.tile([128, qs], FP32, tag="out", name=f"out_ps{cb}")
                for cb in range(n_cb)
            ]
            for jb in range(n_jb):
                jsl = slice(jb * 128, (jb + 1) * 128)
                ps = ps_sc.tile([128, qs], FP32, tag="sc", name="ps")
                for cb in range(n_cb):
                    nc.tensor.matmul(
                        ps[:], kt[cb][:, jsl], kt[cb][:, nsl],
                        start=(cb == 0), stop=(cb == n_cb - 1),
                    )
                e = e_p.tile([128, qs], FP16, tag="e", name="e")
                nc.scalar.activation(
                    e[:], ps[:], mybir.ActivationFunctionType.Exp,
                    bias=bias_e[:], scale=rstd[:, jb:jb + 1],
                )
                if jb == 0:
                    nc.vector.tensor_copy(acc[:], e[:])
                else:
                    nc.vector.tensor_add(acc[:], acc[:], e[:])
                for cb in range(n_cb):
                    nc.tensor.matmul(
                        out_ps[cb][:], km[jb][:, cb * 128:(cb + 1) * 128], e[:],
                        start=(jb == 0), stop=(jb == n_jb - 1),
                    )
            nc.tensor.matmul(sum_ps[:], ones32[:], acc[:], start=True, stop=True)
            srow = ssb_p.tile([1, qs], FP32, tag="srow", name="srow")
            nc.vector.tensor_copy(srow[:], sum_ps[:])
            nc.sync.dma_start(out=s_d[0:1, nsl], in_=srow[:])
            for cb in range(n_cb):
                osb = osb_p.tile([128, qs], FP32, tag="osb", name="osb")
                nc.vector.tensor_copy(osb[:], out_ps[cb][:])
                nc.sync.dma_start(out=o_d[cb * 128:(cb + 1) * 128, nsl], in_=osb[:])

    nc.compile()
    return nc


_CACHE = {}


def _get_program():
    if "nc" not in _CACHE:
        _CACHE["nc"] = build_program()
    return _CACHE["nc"]


def _get_runner():
    """Cached sharded executable over 8 cores (same program/plugin as
    run_bass_kernel_spmd's axon path, but without per-call retracing)."""
    if "runner" in _CACHE:
        return _CACHE["runner"]
    import jax
    from jax.sharding import Mesh, NamedSharding, PartitionSpec
    from jax.experimental.shard_map import shard_map
    from concourse import bass2jax, mybir
    from concourse.bass2jax import _bass_exec_p, partition_id_tensor

    nc = _get_program()
    bass2jax.install_neuronx_cc_hook()
    pname = nc.partition_id_tensor.name if nc.partition_id_tensor else None

    in_names, out_names, out_avals = [], [], []
    for alloc in nc.m.functions[0].allocations:
        if not isinstance(alloc, mybir.MemoryLocationSet):
            continue
        name = alloc.memorylocations[0].name
        if alloc.kind == "ExternalInput":
            if name != pname:
                in_names.append(name)
        elif alloc.kind == "ExternalOutput":
            out_names.append(name)
            out_avals.append(
                jax.core.ShapedArray(
                    tuple(alloc.tensor_shape), mybir.dt.np(alloc.dtype)
                )
            )
    n_params, n_outs = len(in_names), len(out_names)
    all_in = in_names + out_names + ([pname] if pname else [])

    def _body(*args):
        operands = list(args)
        if pname is not None:
            operands.append(partition_id_tensor())
        return tuple(_bass_exec_p.bind(
            *operands, out_avals=tuple(out_avals), in_names=tuple(all_in),
            out_names=tuple(out_names), lowering_input_output_aliases=(),
            sim_require_finite=True, sim_require_nnan=True, nc=nc,
        ))

    devices = jax.devices()[:8]
    mesh = Mesh(np.asarray(devices), ("core",))
    spec = NamedSharding(mesh, PartitionSpec("core"))
    fn = jax.jit(
        shard_map(
            _body, mesh=mesh,
            in_specs=(PartitionSpec("core"),) * (n_params + n_outs),
            out_specs=(PartitionSpec("core"),) * n_outs,
            check_rep=False,
        ),
        donate_argnums=tuple(range(n_params, n_params + n_outs)),
        keep_unused=True,
    )
    zero_host = [
        np.zeros((8 * a.shape[0], *a.shape[1:]), a.dtype) for a in out_avals
    ]

    def run(in_maps):
        concat_in = [
            np.concatenate([np.asarray(m[name]) for m in in_maps], axis=0)
            for name in in_names
        ]
        zeros = [jax.device_put(z, spec) for z in zero_host]
        out = fn(*concat_in, *zeros)
        return [
            {
                name: np.asarray(out[i]).reshape(8, *out_avals[i].shape)[c]
                for i, name in enumerate(out_names)
            }
            for c in range(8)
        ]

    _CACHE["runner"] = run
    return run


def make_in_maps(foreground, mask):
    """Per-core host-side input prep (permute so own half is first)."""
    bs, ch, h, w = foreground.shape
    hw = h * w
    half = hw // 2
    f = np.ascontiguousarray(foreground.reshape(bs, ch, hw), dtype=np.float32)
    m = np.ascontiguousarray(mask.reshape(bs, hw), dtype=np.float32)
    in_maps = []
    for b in range(bs):
        k = f[b] + np.float32(1e-7)                 # [ch, hw], reference's +1e-7
        rstd = 1.0 / np.sqrt((k * k).sum(axis=0, dtype=np.float64))  # [hw]
        rstd = rstd.astype(np.float32)
        f16 = f[b].astype(np.float16)               # [ch, hw]
        km16 = ((rstd * m[b])[:, None] * k.T).astype(np.float16)  # [hw, ch]
        for hh in range(2):
            if hh == 0:
                fc, kmc, rc = f16, km16, rstd
            else:  # swap the two column-halves so own half comes first
                fc = np.concatenate([f16[:, half:], f16[:, :half]], axis=1)
                kmc = np.concatenate([km16[half:], km16[:half]], axis=0)
                rc = np.concatenate([rstd[half:], rstd[:half]])
            in_maps.append({
                "f16": np.ascontiguousarray(fc),
                "km16": np.ascontiguousarray(kmc),
                "rstd": np.ascontiguousarray(rc.reshape(hw // 128, 128).T),
            })
    return in_maps


def kernel(foreground, mask):
    foreground = np.asarray(foreground, dtype=np.float32)
    mask = np.asarray(mask, dtype=np.float32)
    bs, ch, h, w = foreground.shape
    hw = h * w

    in_maps = make_in_maps(foreground, mask)
    try:
        results = _get_runner()(in_maps)
    except Exception:
        # robust fallback: the generic SPMD entry point
        res = run_bass_kernel_spmd(_get_program(), in_maps, list(range(8)))
        results = res.results

    fmap = np.empty((bs, ch, h, w), dtype=np.float32)
    rows = h // 2
    for core in range(8):
        b, hh = core // 2, core % 2
        o = results[core]["o"]       # [ch, hw/2] unnormalized
        s = results[core]["s"]       # [1, hw/2] softmax denominator
        fmap[b, :, hh * rows:(hh + 1) * rows, :] = (o / s).reshape(ch, rows, w)

    mm = mask[:, 0:1]                    # [bs, 1, h, w]
    final = fmap * (1.0 - mm) + foreground * mm
    skip = mask.sum(axis=(1, 2, 3)) > (hw - 10)
    final[skip] = foreground[skip]
    return final.astype(np.float32)


# revision 24
# speedup vs baseline: 672.7314x; 1.0780x over previous
"""ContextualAttentionMask Trainium2 kernel.

Math (per batch sample):
  f: [256, 4096] feature map (channels x pixels), m: [4096] mask
  K[j, :]    = f[:, j] + 1e-7          (per-pixel 1x1 kernel)
  rstd[j]    = 1 / ||K[j, :]||_2
  raw[j, n]  = sum_c f[c, j] * f[c, n]          (only interior columns matter:
               the conv padding columns are dead compute - 1x1 kernels, the
               output at pad positions is cropped, softmax is per-column)
  att[j, n]  = softmax_j(rstd[j] * raw[j, n])
  fmap[c, n] = sum_j rstd[j] * m[j] * K[j, c] * att[j, n]
  final      = fmap * (1 - m) + f * m  ;  skip branch if mask nearly all-ones

Device computes (per core, unnormalized; host divides, blends, skip-branch):
  E[j, n] = exp(rstd[j] * raw[j, n] - 12)       (-12 keeps E in fp16 range;
                                                 cancels in the division)
  o[c, n] = sum_j km16[j, c] * E[j, n]     with km16 = fp16(rstd * m * K)
  s[n]    = sum_j E[j, n]

Sharding: 8 cores = 4 samples x 2 column-halves (2048 columns each).
Inputs are host-permuted so each core's own half is always columns 0..2047;
the j (softmax/contraction) order is irrelevant as long as f16/km16/rstd
agree. Tiny per-j scalars (rstd, rstd*m) and fp16 casts are host-side prep;
all heavy compute (2x 2048x4096x256 GEMMs + softmax) runs on device.
"""

import sys
from contextlib import ExitStack

import numpy as np

sys.path.insert(0, "/opt/trn_rl_repo")

from concourse import bacc, mybir, tile  # noqa: E402
from concourse.bass_utils import run_bass_kernel_spmd  # noqa: E402

FP32 = mybir.dt.float32
FP16 = mybir.dt.float16

CH = 256          # channels
J = 4096          # number of per-pixel kernels (= h*w)
NH = 2048         # columns handled per core (half of a sample)
EXP_BIAS = -12.0  # exp(x - 12) keeps values in fp16 range; cancels on host


def build_program(ch=CH, j_total=J, n_half=NH, bufs_sc=4, bufs_out=4,
                  bufs_e=13, loop_reps=1):
    """Emit the per-core Bass/Tile program (SPMD across 8 cores)."""
    assert ch % 128 == 0 and j_total % 128 == 0
    n_cb = ch // 128          # channel blocks
    n_jb = j_total // 128     # j blocks
    qs = min(512, n_half)     # output column chunk width
    nq = n_half // qs
    assert n_half % qs == 0

    nc = bacc.Bacc("TRN2", target_bir_lowering=False, debug=False, num_devices=8)

    f_d = nc.dram_tensor("f16", [ch, j_total], FP16, kind="ExternalInput").ap()
    km_d = nc.dram_tensor("km16", [j_total, ch], FP16, kind="ExternalInput").ap()
    rstd_d = nc.dram_tensor("rstd", [128, n_jb], FP32, kind="ExternalInput").ap()
    o_d = nc.dram_tensor("o", [ch, n_half], FP32, kind="ExternalOutput").ap()
    s_d = nc.dram_tensor("s", [1, n_half], FP32, kind="ExternalOutput").ap()

    with tile.TileContext(nc) as tc, ExitStack() as ctx:
        const_p = ctx.enter_context(tc.tile_pool(name="const", bufs=1))
        kt_p = ctx.enter_context(tc.tile_pool(name="kt", bufs=n_cb))
        km_p = ctx.enter_context(tc.tile_pool(name="km", bufs=n_jb))
        e_p = ctx.enter_context(tc.tile_pool(name="e", bufs=bufs_e))
        osb_p = ctx.enter_context(tc.tile_pool(name="osb", bufs=3))
        ssb_p = ctx.enter_context(tc.tile_pool(name="ssb", bufs=2))
        ps_sc = ctx.enter_context(
            tc.tile_pool(name="ps_sc", bufs=bufs_sc, space="PSUM"))
        ps_out = ctx.enter_context(
            tc.tile_pool(name="ps_out", bufs=bufs_out, space="PSUM"))

        ones32 = const_p.tile([128, 1], FP32)
        nc.vector.memset(ones32[:], 1.0)
        bias_e = const_p.tile([128, 1], FP32, tag="bias_e")
        nc.vector.memset(bias_e[:], EXP_BIAS)
        rstd = const_p.tile([128, n_jb], FP32, tag="rstd")
        nc.sync.dma_start(out=rstd[:], in_=rstd_d[:, :])

        # fp16 feature map, [c, j] layout; chunked DMA so matmuls start early
        kt = [
            kt_p.tile([128, j_total], FP16, tag="kt", name=f"kt{cb}")
            for cb in range(n_cb)
        ]
        if j_total >= 4096:  # small first chunks so the first matmuls start early
            bounds = [0, 512, 1024, 2048, j_total]
        else:
            bounds = list(range(0, j_total + 1, min(512, j_total)))
        for q8, q9 in zip(bounds[:-1], bounds[1:]):
            for cb in range(n_cb):
                nc.sync.dma_start(
                    out=kt[cb][:, q8:q9],
                    in_=f_d[cb * 128:(cb + 1) * 128, q8:q9],
                )

        # mask-and-norm-scaled kernels, [j, c] layout
        km = []
        for jb in range(n_jb):
            t = km_p.tile([128, ch], FP16, tag="km", name=f"km{jb}")
            nc.sync.dma_start(out=t[:], in_=km_d[jb * 128:(jb + 1) * 128, :])
            km.append(t)

        # fused main loop: scores -> exp -> sumexp & Km^T E accumulation.
        # The softmax denominator is folded partition-wise on the (idle) DVE
        # (acc[p, n] = sum_jb E[jb*128+p, n]); one fp32 ones-matmul per chunk
        # does the final 128-way fold, keeping the PE stream count minimal.
        # loop_reps > 1 repeats the identical work (timing experiments only).
        for q in [qq for _ in range(loop_reps) for qq in range(nq)]:
            nsl = slice(q * qs, (q + 1) * qs)
            sum_ps = ps_out.tile([1, qs], FP32, tag="out", name="sum_ps")
            acc = ssb_p.tile([128, qs], FP32, tag="acc", name="acc")
            out_ps = [
                ps_out.tile([128, qs], FP32, tag="out", name=f"out_ps{cb}")
                for cb in range(n_cb)
            ]
            # software pipeline: the exp-dependent matmuls trail the score
            # matmuls by D j-blocks, so the in-order PE queue never waits on
            # the ACT exp latency (recovers ~6 us of 117 ns/jb stalls).
            D = min(3, n_jb - 1)
            etiles = {}
            for jj in range(n_jb + D):
                if jj < n_jb:
                    jb = jj
                    jsl = slice(jb * 128, (jb + 1) * 128)
                    ps = ps_sc.tile([128, qs], FP32, tag="sc", name="ps")
                    for cb in range(n_cb):
                        nc.tensor.matmul(
                            ps[:], kt[cb][:, jsl], kt[cb][:, nsl],
                            start=(cb == 0), stop=(cb == n_cb - 1),
                        )
                    e = e_p.tile([128, qs], FP16, tag="e", name="e")
                    nc.scalar.activation(
                        e[:], ps[:], mybir.ActivationFunctionType.Exp,
                        bias=bias_e[:], scale=rstd[:, jb:jb + 1],
                    )
                    etiles[jb] = e
                if jj >= D:
                    jb = jj - D
                    e = etiles.pop(jb)
                    if jb == 0:
                        nc.vector.tensor_copy(acc[:], e[:])
                    else:
                        nc.vector.tensor_add(acc[:], acc[:], e[:])
                    for cb in range(n_cb):
                        nc.tensor.matmul(
                            out_ps[cb][:], km[jb][:, cb * 128:(cb + 1) * 128], e[:],
                            start=(jb == 0), stop=(jb == n_jb - 1),
                        )
            nc.tensor.matmul(sum_ps[:], ones32[:], acc[:], start=True, stop=True)
            srow = ssb_p.tile([1, qs], FP32, tag="srow", name="srow")
            nc.vector.tensor_copy(srow[:], sum_ps[:])
            nc.sync.dma_start(out=s_d[0:1, nsl], in_=srow[:])
            for cb in range(n_cb):
                osb = osb_p.tile([128, qs], FP32, tag="osb", name="osb")
                nc.vector.tensor_copy(osb[:], out_ps[cb][:])
                nc.sync.dma_start(out=o_d[cb * 128:(cb + 1) * 128, nsl], in_=osb[:])

    nc.compile()
    return nc


_CACHE = {}


def _get_program():
    if "nc" not in _CACHE:
        _CACHE["nc"] = build_program()
    return _CACHE["nc"]


def _get_runner():
    """Cached sharded executable over 8 cores (same program/plugin as
    run_bass_kernel_spmd's axon path, but without per-call retracing)."""
    if "runner" in _CACHE:
        return _CACHE["runner"]
    import jax
    from jax.sharding import Mesh, NamedSharding, PartitionSpec
    from jax.experimental.shard_map import shard_map
    from concourse import bass2jax, mybir
    from concourse.bass2jax import _bass_exec_p, partition_id_tensor

    nc = _get_program()
    bass2jax.install_neuronx_cc_hook()
    pname = nc.partition_id_tensor.name if nc.partition_id_tensor else None

    in_names, out_names, out_avals = [], [], []
    for alloc in nc.m.functions[0].allocations:
        if not isinstance(alloc, mybir.MemoryLocationSet):
            continue
        name = alloc.memorylocations[0].name
        if alloc.kind == "ExternalInput":
            if name != pname:
                in_names.append(name)
        elif alloc.kind == "ExternalOutput":
            out_names.append(name)
            out_avals.append(
                jax.core.ShapedArray(
                    tuple(alloc.tensor_shape), mybir.dt.np(alloc.dtype)
                )
            )
    n_params, n_outs = len(in_names), len(out_names)
    all_in = in_names + out_names + ([pname] if pname else [])

    def _body(*args):
        operands = list(args)
        if pname is not None:
            operands.append(partition_id_tensor())
        return tuple(_bass_exec_p.bind(
            *operands, out_avals=tuple(out_avals), in_names=tuple(all_in),
            out_names=tuple(out_names), lowering_input_output_aliases=(),
            sim_require_finite=True, sim_require_nnan=True, nc=nc,
        ))

    devices = jax.devices()[:8]
    mesh = Mesh(np.asarray(devices), ("core",))
    spec = NamedSharding(mesh, PartitionSpec("core"))
    fn = jax.jit(
        shard_map(
            _body, mesh=mesh,
            in_specs=(PartitionSpec("core"),) * (n_params + n_outs),
            out_specs=(PartitionSpec("core"),) * n_outs,
            check_rep=False,
        ),
        donate_argnums=tuple(range(n_params, n_params + n_outs)),
        keep_unused=True,
    )
    zero_host = [
        np.zeros((8 * a.shape[0], *a.shape[1:]), a.dtype) for a in out_avals
    ]

    def run(in_maps):
        concat_in = [
            np.concatenate([np.asarray(m[name]) for m in in_maps], axis=0)
            for name in in_names
        ]
        zeros = [jax.device_put(z, spec) for z in zero_host]
        out = fn(*concat_in, *zeros)
        return [
            {
                name: np.asarray(out[i]).reshape(8, *out_avals[i].shape)[c]
                for i, name in enumerate(out_names)
            }
            for c in range(8)
        ]

    _CACHE["runner"] = run
    return run


def make_in_maps(foreground, mask):
    """Per-core host-side input prep (permute so own half is first)."""
    bs, ch, h, w = foreground.shape
    hw = h * w
    half = hw // 2
    f = np.ascontiguousarray(foreground.reshape(bs, ch, hw), dtype=np.float32)
    m = np.ascontiguousarray(mask.reshape(bs, hw), dtype=np.float32)
    in_maps = []
    for b in range(bs):
        k = f[b] + np.float32(1e-7)                 # [ch, hw], reference's +1e-7
        rstd = 1.0 / np.sqrt((k * k).sum(axis=0, dtype=np.float64))  # [hw]
        rstd = rstd.astype(np.float32)
        f16 = f[b].astype(np.float16)               # [ch, hw]
        km16 = ((rstd * m[b])[:, None] * k.T).astype(np.float16)  # [hw, ch]
        for hh in range(2):
            if hh == 0:
                fc, kmc, rc = f16, km16, rstd
            else:  # swap the two column-halves so own half comes first
                fc = np.concatenate([f16[:, half:], f16[:, :half]], axis=1)
                kmc = np.concatenate([km16[half:], km16[:half]], axis=0)
                rc = np.concatenate([rstd[half:], rstd[:half]])
            in_maps.append({
                "f16": np.ascontiguousarray(fc),
                "km16": np.ascontiguousarray(kmc),
                "rstd": np.ascontiguousarray(rc.reshape(hw // 128, 128).T),
            })
    return in_maps


def kernel(foreground, mask):
    foreground = np.asarray(foreground, dtype=np.float32)
    mask = np.asarray(mask, dtype=np.float32)
    bs, ch, h, w = foreground.shape
    hw = h * w

    in_maps = make_in_maps(foreground, mask)
    try:
        results = _get_runner()(in_maps)
    except Exception:
        # robust fallback: the generic SPMD entry point
        res = run_bass_kernel_spmd(_get_program(), in_maps, list(range(8)))
        results = res.results

    fmap = np.empty((bs, ch, h, w), dtype=np.float32)
    rows = h // 2
    for core in range(8):
        b, hh = core // 2, core % 2
        o = results[core]["o"]       # [ch, hw/2] unnormalized
        s = results[core]["s"]       # [1, hw/2] softmax denominator
        fmap[b, :, hh * rows:(hh + 1) * rows, :] = (o / s).reshape(ch, rows, w)

    mm = mask[:, 0:1]                    # [bs, 1, h, w]
    final = fmap * (1.0 - mm) + foreground * mm
    skip = mask.sum(axis=(1, 2, 3)) > (hw - 10)
    final[skip] = foreground[skip]
    return final.astype(np.float32)
